# revision 3
# baseline (speedup 1.0000x reference)
"""Trainium2 Bass kernel for nn_BOREP (dense_mlp):

    out[s, b, o] = einsum('sbi,oi->sbo', x, W) + bias[o]
    x [256, 64, 1024] f32, W [4096, 1024] f32, bias [4096] f32 -> out [256, 64, 4096] f32

Strategy
--------
Data-parallel over 8 NeuronCores: shard x along seq (axis 0), 32 timesteps per
core, i.e. per-core A = x-shard reshaped to [2048, 1024]; W and bias
replicated. Per core: out_shard = A @ W.T + bias -> [2048, 4096].

Per-core numeric scheme ("all-fp8 DoubleRow, 3 terms, unit PSUM scale"):
the rel-err budget is 2e-2; exact numpy simulation of the fixed problem inputs
shows a 3-term fp8 expansion lands at 1.3e-3, so the whole product runs at the
PE's fp8 DoubleRow rate (0.5 cyc/row, 2 k-tiles per instruction):

    x8  = e4m3(x),      dx8 = e4m3(x - x8)     (x-side residual, e4m3)
    w8  = e4m3(W),      dw5 = e5m2(W - w8)     (W-side residual, e5m2: its wide
                                                exponent range reaches the
                                                ~2^-10..2^-6 residual scale that
                                                e4m3 cannot represent at scale 1)
    A @ W.T ~= x8 @ w8.T + dx8 @ w8.T + x8 @ dw5.T   (dx8 @ dw5.T ~ 2^-16, dropped)

All three terms share PSUM scale 1, so they accumulate into a single PSUM bank
and the result needs no rescaling: one DVE tensor_tensor per tile adds the
bias during the PSUM->SBUF copy. PE cost 1.5 cyc/row vs 2.56 for the previous
f32r+fp8 scheme and 4 for native fp32: 12 DoubleRow matmuls per [128, 512]
output tile = 3072 PE cycles = 1.28 us; 128 tiles ~= 164 us/core at 2.4 GHz.

Layout: host pre-blocks operands so every DMA lands [128, kt, free] tiles with
>=1KB-contiguous runs per partition; contraction dim k on SBUF partitions.
Loop is n-outer with the x-side SBUF-resident (~32KB/partition) and W streamed
once (8MB total traffic), double-buffered. Input DMAs ride the ACT queue,
bias + output DMAs the SP queue, so in/out transfers overlap on the bus.
"""
import sys

if "/opt/trn_rl_repo" not in sys.path:
    sys.path.insert(0, "/opt/trn_rl_repo")

import numpy as np
import ml_dtypes

# Problem constants (hardcoded per contest contract)
SEQ, BATCH, IN_DIM, OUT_DIM = 256, 64, 1024, 4096
N_CORES = 8
P = 128
K = IN_DIM
M = SEQ * BATCH // N_CORES     # 2048 rows per core
N = OUT_DIM
KT = K // P                    # 8 k-tiles
TM = 128                       # out-tile rows (PSUM partitions)
TN = 512                       # out-tile cols (one PSUM bank of fp32)
MT = M // TM                   # 16
NT = N // TN                   # 8

E4M3 = ml_dtypes.float8_e4m3
E5M2 = ml_dtypes.float8_e5m2

_cache = {}


def _build_nc(repeat: int = 1):
    import concourse.mybir as mybir
    import concourse.tile as tile
    from concourse import bacc
    from contextlib import ExitStack

    F32 = mybir.dt.float32
    F8 = mybir.dt.float8e4
    F85 = mybir.dt.float8e5

    nc = bacc.Bacc("TRN2", target_bir_lowering=False, debug=False)

    x8 = nc.dram_tensor("x8", [MT, P, KT, TM], F8, kind="ExternalInput").ap()
    dx8 = nc.dram_tensor("dx8", [MT, P, KT, TM], F8, kind="ExternalInput").ap()
    w8 = nc.dram_tensor("w8", [NT, P, KT, TN], F8, kind="ExternalInput").ap()
    dw5 = nc.dram_tensor("dw5", [NT, P, KT, TN], F85, kind="ExternalInput").ap()
    bias = nc.dram_tensor("bias", [P, N], F32, kind="ExternalInput").ap()
    out = nc.dram_tensor("out", [M, N], F32, kind="ExternalOutput").ap()

    with tile.TileContext(nc) as tc:
        with ExitStack() as ctx:
            xpool = ctx.enter_context(tc.tile_pool(name="xpool", bufs=1))
            wpool = ctx.enter_context(tc.tile_pool(name="wpool", bufs=2))
            opool = ctx.enter_context(tc.tile_pool(name="opool", bufs=8))
            cpool = ctx.enter_context(tc.tile_pool(name="cpool", bufs=1))
            ps = ctx.enter_context(tc.tile_pool(name="ps", bufs=6, space="PSUM"))

            bias_sb = cpool.tile([P, N], F32)

            for _ in range(repeat):
                # DMA emission order = consumption order. Inputs go on the DVE
                # queue; bias + outputs on the SP queue (bias first so the DVE
                # engine's first tensor_tensor isn't blocked behind x loads).
                x8_sb, dx_sb = [], []

                # First operands in fine grain: x8[0] whole, w8[0] per k-tile
                # (first matmul starts after one 64KB chunk), then the rest.
                t1 = xpool.tile([P, KT, TM], F8, tag="x8_0")
                nc.scalar.dma_start(t1[:], x8[0])
                x8_sb.append(t1)
                w0 = wpool.tile([P, KT, TN], F8, tag="w8")
                for k in range(KT):
                    nc.scalar.dma_start(w0[:, k], w8[0, :, k])
                t2 = xpool.tile([P, KT, TM], F8, tag="dx_0")
                nc.scalar.dma_start(t2[:], dx8[0])
                dx_sb.append(t2)
                d0 = wpool.tile([P, KT, TN], F85, tag="dw")
                nc.scalar.dma_start(d0[:], dw5[0])
                nc.sync.dma_start(bias_sb[:], bias[:])
                for m in range(1, MT):
                    t1 = xpool.tile([P, KT, TM], F8, tag=f"x8_{m}")
                    nc.scalar.dma_start(t1[:], x8[m])
                    t2 = xpool.tile([P, KT, TM], F8, tag=f"dx_{m}")
                    nc.scalar.dma_start(t2[:], dx8[m])
                    x8_sb.append(t1)
                    dx_sb.append(t2)

                for n in range(NT):
                    if n == 0:
                        w_sb, d_sb = w0, d0
                    else:
                        w_sb = wpool.tile([P, KT, TN], F8, tag="w8")
                        nc.scalar.dma_start(w_sb[:], w8[n])
                        d_sb = wpool.tile([P, KT, TN], F85, tag="dw")
                        nc.scalar.dma_start(d_sb[:], dw5[n])
                    # DoubleRow: [P, KT, X] viewed as [P, KT//2, 2, X];
                    # each instruction contracts 2 k-tiles (256 values).
                    wv = w_sb.rearrange("p (j i) t -> p j i t", i=2)
                    dv = d_sb.rearrange("p (j i) t -> p j i t", i=2)

                    for m in range(MT):
                        xv = x8_sb[m].rearrange("p (j i) t -> p j i t", i=2)
                        dxv = dx_sb[m].rearrange("p (j i) t -> p j i t", i=2)
                        pm = ps.tile([P, TN], F32)
                        i = 0
                        for (lv, rv) in ((xv, wv), (dxv, wv), (xv, dv)):
                            for j in range(KT // 2):
                                nc.tensor.matmul(
                                    pm[:], lv[:, j], rv[:, j],
                                    start=(i == 0), stop=(i == 3 * (KT // 2) - 1),
                                    perf_mode=mybir.MatmulPerfMode.DoubleRow,
                                )
                                i += 1
                        o_sb = opool.tile([P, TN], F32)
                        nc.vector.tensor_tensor(
                            o_sb[:], pm[:], bias_sb[:, n * TN:(n + 1) * TN],
                            mybir.AluOpType.add)
                        nc.sync.dma_start(
                            out[m * TM:(m + 1) * TM, n * TN:(n + 1) * TN], o_sb[:]
                        )
    nc.compile()
    return nc


def get_nc():
    if "nc" not in _cache:
        _cache["nc"] = _build_nc()
    return _cache["nc"]


def _blk_x(a2d):
    """[M, K] -> [MT, P, KT, TM] with blk[m, p, k, j] = a2d[m*TM+j, k*P+p]."""
    aT = np.ascontiguousarray(a2d.T)  # [K, M]
    return np.ascontiguousarray(aT.reshape(KT, P, MT, TM).transpose(2, 1, 0, 3))


def _blk_w(wt):
    """[K, N] -> [NT, P, KT, TN] with blk[n, p, k, j] = wt[k*P+p, n*TN+j]."""
    return np.ascontiguousarray(wt.reshape(KT, P, NT, TN).transpose(2, 1, 0, 3))


def prep_in_maps(x, W, b):
    x = np.asarray(x, dtype=np.float32)
    W = np.asarray(W, dtype=np.float32)
    b = np.asarray(b, dtype=np.float32)

    A = x.reshape(SEQ * BATCH, K)
    w8 = W.astype(E4M3)
    dw5 = (W - w8.astype(np.float32)).astype(E5M2)
    w8b = _blk_w(np.ascontiguousarray(w8.T))
    dw5b = _blk_w(np.ascontiguousarray(dw5.T))
    bias_bcast = np.ascontiguousarray(np.broadcast_to(b, (P, N)))

    in_maps = []
    for c in range(N_CORES):
        Ac = A[c * M:(c + 1) * M]
        x8 = Ac.astype(E4M3)
        dx8 = (Ac - x8.astype(np.float32)).astype(E4M3)
        in_maps.append({
            "x8": _blk_x(x8),
            "dx8": _blk_x(dx8),
            "w8": w8b, "dw5": dw5b, "bias": bias_bcast,
        })
    return in_maps


def kernel(x, W, b):
    from concourse.bass_utils import run_bass_kernel_spmd

    in_maps = prep_in_maps(x, W, b)
    nc = get_nc()
    res = run_bass_kernel_spmd(nc, in_maps, core_ids=list(range(N_CORES)))
    full = np.concatenate([r["out"] for r in res.results], axis=0)
    return full.reshape(SEQ, BATCH, OUT_DIM).astype(np.float32)
